# revision 23
# baseline (speedup 1.0000x reference)
"""Trainium2 Bass kernel for nn_Attention_867583394433 (sparse window attention).

Strategy (8 NeuronCores, pure data parallel over windows B_=256 -> 32/core):
  - Host precomputes the position-MLP -> relative-position-bias table, folds it
    with the additive mask into a multiplicative table EM = exp(rpb + mask)
    (fp16), resident in SBUF (8 masks/core).
  - Device, per window, in transposed score layout S^T[m, n]:
      qk^T + v matmuls -> exp on ScalarE -> P = exp(S^T) * EM split across
      VectorE and GpSimd -> flipped PV (P as stationary operand, out [n, d])
      with the softmax denominator as 1-wide matmuls -> reciprocal + broadcast
      normalize on VectorE -> transpose attn-out back to [c, n] (DMA xbar for
      c 0:128, PE transpose for c 128:192) -> output projection -> DMA out.
  - Biases folded via ones rows; q-scale folded into w_q on the host.
"""

import os

import numpy as np

HEADS = 6
D = 32
C = 192
N = 256
B = 256
NMASK = 64
POS_DIM = 12
EPS = 1e-5
NCORES = 8
WPC = B // NCORES  # 32 windows per core
MPC = NMASK // NCORES  # 8 masks per core
REP = B // NMASK  # 4 windows sharing one mask
FREE = HEADS * 2 * N  # 3072: free layout (head, mtile, n)

# elems of the P=exp(S)*EM multiply done on GpSimd (leading; rest on VectorE)
PM_POOL = int(os.environ.get("PM_POOL", "1536"))

_CACHE = {}


def _win_to_b(core, w):
    """Window order within a core: mask-major.  w = j*REP + k  ->  b."""
    j, k = divmod(w, REP)
    return NMASK * k + MPC * core + j


def _ln_np(x, g, b):
    m = x.mean(-1, keepdims=True)
    v = x.var(-1, keepdims=True)
    return (x - m) / np.sqrt(v + EPS) * g + b


def _pos_bias_host(H, W, pw0, pb0, g1, be1, w1, b1, g2, be2, w2, b2, g3, be3, w3, b3):
    """Replicates the reference position MLP + gather -> rpb [N, N, HEADS]."""
    H = int(H)
    W = int(W)
    ph = np.arange(1 - H, H)
    pw = np.arange(1 - W, W)
    biases = (
        np.stack(np.meshgrid(ph, pw, indexing="ij")).reshape(2, -1).T.astype(np.float32)
    )
    pos = biases @ pw0 + pb0
    pos = np.maximum(_ln_np(pos, g1, be1), 0.0) @ w1 + b1
    pos = np.maximum(_ln_np(pos, g2, be2), 0.0) @ w2 + b2
    pos = np.maximum(_ln_np(pos, g3, be3), 0.0) @ w3 + b3
    coords = np.stack(np.meshgrid(np.arange(H), np.arange(W), indexing="ij")).reshape(
        2, -1
    )
    rel = coords[:, :, None] - coords[:, None, :]
    rpi = (rel[0] + H - 1) * (2 * W - 1) + (rel[1] + W - 1)
    return pos[rpi]  # [N, N, HEADS] fp32


def _build_nc(repeat=1):
    import concourse.tile as tile
    from concourse import bacc, mybir

    FP = mybir.dt.float32
    BF = mybir.dt.float16
    EXP = mybir.ActivationFunctionType.Exp
    MUL = mybir.AluOpType.mult

    nc = bacc.Bacc("TRN2", target_bir_lowering=False, debug=False)
    xt_d = nc.dram_tensor("xt", [WPC, 128, 2, N], BF, kind="ExternalInput")
    em_d = nc.dram_tensor("em", [MPC, 128, FREE], BF, kind="ExternalInput")
    wqk_d = nc.dram_tensor("wqk", [193, 512], BF, kind="ExternalInput")
    wv_d = nc.dram_tensor("wv", [193, C], BF, kind="ExternalInput")
    wp_d = nc.dram_tensor("wp", [193, C], BF, kind="ExternalInput")
    y_d = nc.dram_tensor("y", [WPC, 128, 2, C], FP, kind="ExternalOutput")

    with tile.TileContext(nc) as tc:
        with (
            tc.tile_pool(name="const", bufs=1) as cpool,
            tc.tile_pool(name="win", bufs=int(os.environ.get("WBUFS", "4"))) as wpool,
            tc.tile_pool(name="big", bufs=int(os.environ.get("BBUFS", "4"))) as bpool,
            tc.tile_pool(name="ps_qk", bufs=2, space="PSUM") as ps_qk,
            tc.tile_pool(name="ps_sc", bufs=2, space="PSUM") as ps_sc,
            tc.tile_pool(name="ps_py", bufs=2, space="PSUM") as ps_py,
        ):
            # ---- resident constants ----
            em_sb = cpool.tile([128, MPC, FREE], BF)
            em_loaded = set()
            wqk_sb = cpool.tile([128, 2, 512], BF)
            nc.sync.dma_start(wqk_sb[:, 0, :], wqk_d[0:128, :])
            nc.sync.dma_start(wqk_sb[0:65, 1, :], wqk_d[128:193, :])
            wv_sb = cpool.tile([128, 2, C], BF)
            nc.sync.dma_start(wv_sb[:, 0, :], wv_d[0:128, :])
            nc.sync.dma_start(wv_sb[0:65, 1, :], wv_d[128:193, :])
            wp_sb = cpool.tile([128, 2, C], BF)
            nc.sync.dma_start(wp_sb[:, 0, :], wp_d[0:128, :])
            nc.sync.dma_start(wp_sb[0:65, 1, :], wp_d[128:193, :])
            ones1 = cpool.tile([128, 1], BF)
            nc.gpsimd.memset(ones1[:], 1.0)
            # attn-out^T ring buffers; row 64 of the hi tile is the ones row
            NAOT = int(os.environ.get("NAOT", "3"))
            aoTa = [cpool.tile([128, 2, 128], BF, name=f"aoTa{k}", tag=f"aoTa{k}") for k in range(NAOT)]
            aoTb = [cpool.tile([128, 2, 128], BF, name=f"aoTb{k}", tag=f"aoTb{k}") for k in range(NAOT)]

            ysb_cur = [None]

            # scores head -> (qk m-tile, partition row) maps (q/k row-aligned)
            q_loc = [(0, 32 * h) for h in range(4)] + [(2, 32 * (h - 4)) for h in (4, 5)]
            k_loc = [(1, 32 * h) for h in range(4)] + [(3, 32 * (h - 4)) for h in (4, 5)]

            def load_pair(p):
                """One DMA loading x^T for windows 2p, 2p+1."""
                xa = wpool.tile([128, 2, 2, N], BF, tag="xa", name=f"xa{p}")
                nc.sync.dma_start(xa[:], xt_d[2 * p : 2 * p + 2].rearrange("w p t n -> p w t n"))
                return xa

            def load_em(j):
                if j not in em_loaded:
                    em_loaded.add(j)
                    nc.sync.dma_start(em_sb[:, j, :], em_d[j])

            def qk_part(w, xa):
                """qk^T matmuls + copies for window w (hoisted one iter early)."""
                x = xa[:, w % 2]
                qkT = wpool.tile([128, 4, N], BF, tag="qkT", name=f"qkT{w}")
                for half in range(2):
                    qkps = ps_qk.tile([128, 2, N], FP, tag="qk", name=f"qkps{w}_{half}")
                    for mm in range(2):
                        m = 2 * half + mm
                        nc.tensor.matmul(
                            qkps[:, mm, :],
                            wqk_sb[:, 0, 128 * m : 128 * (m + 1)],
                            x[:, 0, :],
                            start=True,
                            stop=False,
                        )
                        nc.tensor.matmul(
                            qkps[:, mm, :],
                            wqk_sb[0:65, 1, 128 * m : 128 * (m + 1)],
                            x[0:65, 1, :],
                            start=False,
                            stop=True,
                        )
                    nc.vector.tensor_copy(
                        qkT[:, 2 * half : 2 * half + 2, :], qkps[:]
                    )
                return {"w": w, "j": w // REP, "asel": w % NAOT, "qkT": qkT, "xa": xa}

            def v_part(st):
                """v matmuls (v in [m, c] layout) for window st, emitted late."""
                w = st["w"]
                x = st["xa"][:, w % 2]
                vps = ps_qk.tile([128, 2, C], FP, tag="qk", name=f"vps{w}")
                for mt in range(2):
                    nc.tensor.matmul(
                        vps[:, mt, :],
                        x[:, 0, 128 * mt : 128 * (mt + 1)],
                        wv_sb[:, 0, :],
                        start=True,
                        stop=False,
                    )
                    nc.tensor.matmul(
                        vps[:, mt, :],
                        x[0:65, 1, 128 * mt : 128 * (mt + 1)],
                        wv_sb[0:65, 1, :],
                        start=False,
                        stop=True,
                    )
                vsb = wpool.tile([128, 2, C], BF, tag="vsb", name=f"vsb{w}")
                nc.vector.tensor_copy(vsb[:], vps[:])
                st["vsb"] = vsb

            def scores(st):
                scores_p1(st)
                scores_p2(st)

            def sc_chunk(st, ch):
                w = st["w"]
                qkT = st["qkT"]
                es = st["es"]
                scps = ps_sc.tile([128, 4, N], FP, tag="sc", name=f"scps{w}_{ch}")
                for kk in range(4):
                    h, mt = divmod(4 * ch + kk, 2)
                    qt, qr = q_loc[h]
                    kt, kr = k_loc[h]
                    nc.tensor.matmul(
                        scps[:, kk, :],
                        qkT[kr : kr + 32, kt, 128 * mt : 128 * (mt + 1)],
                        qkT[qr : qr + 32, qt, :],
                        start=True,
                        stop=True,
                        tile_position=(kr, 0),
                    )
                nc.scalar.activation(es[:, 1024 * ch : 1024 * (ch + 1)], scps[:], EXP)

            def scores_p1(st):
                st["es"] = bpool.tile([128, FREE], BF, tag="es", name=f"es{st['w']}")
                sc_chunk(st, 0)
                sc_chunk(st, 1)

            def scores_p2(st):
                sc_chunk(st, 2)

            def pm_alloc(st):
                p_t = bpool.tile([128, FREE], BF, tag="P", name=f"P{st['w']}")
                st["p"] = p_t
                return p_t, em_sb[:, st["j"], :]

            def pm_pool(st):
                p_t, emj = pm_alloc(st)
                for lo, hi in ((0, 1024), (1024, PM_POOL)):
                    if lo < hi:
                        nc.gpsimd.tensor_tensor(
                            p_t[:, lo:hi], st["es"][:, lo:hi], emj[:, lo:hi], MUL
                        )

            def pm_dve(st, lo, hi):
                p_t = st["p"]
                emj = em_sb[:, st["j"], :]
                nc.vector.tensor_tensor(
                    p_t[:, lo:hi], st["es"][:, lo:hi], emj[:, lo:hi], MUL
                )

            def pv_part(st, heads):
                """flipped PV + denominator for a subset of heads."""
                p_t = st["p"]
                vsb = st["vsb"]
                if "pv" not in st:
                    st["pv"] = ps_py.tile(
                        [128, 2, 6, 33], FP, tag="py", name=f"pv{st['w']}"
                    )
                pv = st["pv"]
                for h in heads:
                    for nt in range(2):
                        # each accumulation group completes before the next:
                        # start=True clears has_written for the whole bank
                        for mt in range(2):
                            o = 512 * h + 256 * mt + 128 * nt
                            nc.tensor.matmul(
                                pv[:, nt, h, 0:32],
                                p_t[:, o : o + 128],
                                vsb[:, mt, 32 * h : 32 * (h + 1)],
                                start=(mt == 0),
                                stop=(mt == 1),
                            )
                        for mt in range(2):
                            o = 512 * h + 256 * mt + 128 * nt
                            nc.tensor.matmul(
                                pv[:, nt, h, 32:33],
                                p_t[:, o : o + 128],
                                ones1[:],
                                start=(mt == 0),
                                stop=(mt == 1),
                            )

            def norm_transpose(st):
                """normalize + launch all xbar transposes for window st."""
                w = st["w"]
                asel = st["asel"]
                pv = st["pv"]
                ivd = wpool.tile([128, 2, 6], FP, tag="ivd", name=f"ivd{w}")
                nc.vector.reciprocal_approx_fast(ivd[:], pv[:, :, :, 32])
                aout = wpool.tile([128, 2, 8, 32], BF, tag="aout", name=f"aout{w}")
                # pad-head slots 6/7 = 1.0: lands the ones row at aoTb row 64
                nc.gpsimd.memset(aout[:, :, 6:8, :], 1.0)
                av = aout[:].rearrange("p t h d -> p t (h d)")
                for nt in range(2):
                    nc.vector.tensor_tensor(
                        aout[:, nt, 0:6],
                        pv[:, nt, :, 0:32],
                        ivd[:, nt].unsqueeze(2).broadcast_to([128, 6, 32]),
                        MUL,
                    )
                    nc.sync.dma_start_transpose(
                        aoTa[asel][:, nt, :], av[:, nt, 0:128]
                    )
                    nc.sync.dma_start_transpose(
                        aoTb[asel][:, nt, :], av[:, nt, 128:256]
                    )

            def proj_store(st, ysb):
                """Output projection + store for a finished window."""
                w = st["w"]
                asel = st["asel"]
                yps = ps_py.tile([128, 2, C], FP, tag="py", name=f"yps{w}")
                for nt in range(2):
                    nc.tensor.matmul(
                        yps[:, nt, :],
                        aoTa[asel][:, nt, :],
                        wp_sb[:, 0, :],
                        start=True,
                        stop=False,
                    )
                    nc.tensor.matmul(
                        yps[:, nt, :],
                        aoTb[asel][0:65, nt, :],
                        wp_sb[0:65, 1, :],
                        start=False,
                        stop=True,
                    )
                nc.scalar.copy(ysb[:, w % 2], yps[:])
                if w % 2 == 1:
                    nc.sync.dma_start(
                        y_d[w - 1 : w + 1].rearrange("w p t c -> p w t c"), ysb[:]
                    )

            # ---- software-pipelined main loop (depth 4) ----
            # iteration it emits: qkv(it+2) | scores(it+1) | pv(it) | proj(it-1)
            load_em(0)
            xa_pairs = {0: load_pair(0), 1: load_pair(1)}

            def qkv(w):
                p = w // 2
                for q in (p, p + 1):
                    if q <= (WPC - 1) // 2 and q not in xa_pairs:
                        xa_pairs[q] = load_pair(q)
                load_em(min(w, WPC - 1) // REP)
                st = qk_part(w, xa_pairs[p])
                v_part(st)
                return st

            sts = {0: qkv(0), 1: qkv(1)}
            scores(sts[0])
            pm_pool(sts[0])
            pm_dve(sts[0], PM_POOL, 2048)
            pm_dve(sts[0], 2048, FREE)
            for it in range(WPC):
                if it + 2 < WPC:
                    sts[it + 2] = qkv(it + 2)
                s1 = sts.get(it + 1)
                s0 = sts[it]
                if s1 is not None:
                    scores_p1(s1)
                    pm_pool(s1)
                pv_part(s0, (0, 1, 2, 3))
                if s1 is not None:
                    scores_p2(s1)
                    pm_dve(s1, PM_POOL, 2048)
                pv_part(s0, (4, 5))
                norm_transpose(s0)
                if s1 is not None:
                    pm_dve(s1, 2048, FREE)
                if it >= 1:
                    sp = sts.pop(it - 1)
                    if sp["w"] % 2 == 0:
                        ysb = wpool.tile(
                            [128, 2, 2, C], FP, tag="ysb", name=f"ysb{sp['w']}"
                        )
                        ysb_cur[0] = ysb
                    proj_store(sp, ysb_cur[0])
            sp = sts.pop(WPC - 1)
            if sp["w"] % 2 == 0:
                ysb_cur[0] = wpool.tile([128, 2, 2, C], FP, tag="ysb", name="ysb_last")
            proj_store(sp, ysb_cur[0])

    nc.compile()
    return nc


def _prep_inputs(inputs):
    x = np.asarray(inputs["x"], np.float32)
    mask = np.asarray(inputs["mask"], np.float32)
    w_qkv = np.asarray(inputs["w_qkv"], np.float32)
    b_qkv = np.asarray(inputs["b_qkv"], np.float32)
    w_proj = np.asarray(inputs["w_proj"], np.float32)
    b_proj = np.asarray(inputs["b_proj"], np.float32)
    H, W = int(inputs["H"]), int(inputs["W"])

    scale = float(D) ** -0.5
    rpb = _pos_bias_host(
        H,
        W,
        *[
            np.asarray(inputs[k], np.float32)
            for k in (
                "pw0",
                "pb0",
                "g1",
                "be1",
                "w1",
                "b1",
                "g2",
                "be2",
                "w2",
                "b2",
                "g3",
                "be3",
                "w3",
                "b3",
            )
        ],
    )

    # EM[mb, p, h*512 + mt*256 + n] = exp(mask[mb, n, m] + rpb[n, m, h]), m = mt*128+p
    bias = mask.transpose(0, 2, 1)[:, None] + rpb.transpose(2, 1, 0)[None]
    em = np.exp(bias)  # [64, 6, 256(m), 256(n)]
    em = em.reshape(NMASK, HEADS, 2, 128, N).transpose(0, 3, 1, 2, 4)
    em = np.ascontiguousarray(em.reshape(NMASK, 128, FREE)).astype(np.float16)

    # packed/augmented weights
    wq = np.vstack([w_qkv[:, 0:C] * scale, (b_qkv[0:C] * scale)[None]])  # [193, 192]
    wk = np.vstack([w_qkv[:, C : 2 * C], b_qkv[C : 2 * C][None]])
    mmdt = np.float16
    wqk = np.zeros((193, 512), np.float32)
    wqk[:, 0:128] = wq[:, 0:128]
    wqk[:, 128:256] = wk[:, 0:128]
    wqk[:, 256:320] = wq[:, 128:192]
    wqk[:, 384:448] = wk[:, 128:192]
    wqk = wqk.astype(mmdt)
    wv = np.ascontiguousarray(np.vstack([w_qkv[:, 2 * C :], b_qkv[2 * C :][None]])).astype(mmdt)
    wp = np.ascontiguousarray(np.vstack([w_proj, b_proj[None]])).astype(mmdt)
    ident = np.eye(128, dtype=mmdt)

    # per-core x^T with ones row, padded to [B, 128, 2, N] for 1-DMA loads
    xt_aug = np.zeros((B, 128, 2, N), mmdt)
    xT = x.transpose(0, 2, 1)  # [B, C, N]
    xt_aug[:, :, 0, :] = xT[:, 0:128, :]
    xt_aug[:, 0:64, 1, :] = xT[:, 128:192, :]
    xt_aug[:, 64, 1, :] = 1.0

    in_maps = []
    for core in range(NCORES):
        bs = [_win_to_b(core, w) for w in range(WPC)]
        in_maps.append(
            {
                "xt": np.ascontiguousarray(xt_aug[bs]),
                "em": np.ascontiguousarray(em[MPC * core : MPC * (core + 1)]),
                "wqk": wqk,
                "wv": wv,
                "wp": wp,
                "ident": ident,
            }
        )
    return in_maps


def _assemble(results):
    out = np.empty((B, N, C), np.float32)
    for core in range(NCORES):
        y = results[core]["y"]  # [WPC, 128, 2, C]
        for w in range(WPC):
            b = _win_to_b(core, w)
            out[b] = y[w].transpose(1, 0, 2).reshape(N, C)
    return out


def run(inputs, trace=False):
    from concourse.bass_utils import run_bass_kernel_spmd

    if "nc" not in _CACHE:
        _CACHE["nc"] = _build_nc()
    in_maps = _prep_inputs(inputs)
    res = run_bass_kernel_spmd(
        _CACHE["nc"],
        in_maps,
        core_ids=list(range(NCORES)),
        trace=trace,
        trace_cores=[0] if trace else None,
    )
    return _assemble(res.results), res


def get_nc():
    if "nc" not in _CACHE:
        _CACHE["nc"] = _build_nc()
    return _CACHE["nc"]


def kernel(**inputs):
    out, _ = run(inputs, trace=bool(int(os.environ.get("KERNEL_TRACE", "0"))))
    return out
